# revision 53
# baseline (speedup 1.0000x reference)
"""GAT single-head forward on 8 Trainium2 NeuronCores (Bass/Tile).

Math (per reference):
    h   = X @ W + b                      [N, 128]
    f1  = h @ v0, f2 = h @ v1            [N]
    logits = adj * (f1[:,None] + f2[None,:])   (adj entries are exactly 0/1)
    vals = sigmoid(logits) - 0.5
    masked softmax over row edges; out = probs @ h

Key identities used on device:
  * On edges (adj==1): val = sigmoid(f1_i + f2_j) - 0.5 in (-0.5, 0.5), so the
    softmax max-subtraction is unnecessary (exp never overflows) and constant
    shifts cancel:  probs = adj*exp(sigmoid(s)) / rowsum(adj*exp(sigmoid(s))).
  * sigmoid(s) = 0.5*tanh(s/2) + 0.5, so exp(sigmoid(s)) = exp(0.5*t + 0.5)
    with t = tanh(s/2). Tanh and Exp live in the SAME activation table set
    ("exp_and_others"), avoiding per-tile ~2.7us table swaps that
    Sigmoid+Exp would incur.
  * A ones-column appended to h turns the softmax denominator into one extra
    matmul output column (no separate row-reduction pass).

Sharding: rows of adj across the 8 cores (1024 rows each). node_feats is
small (8 MB) and is replicated, so every core computes the full projected
h locally - no collectives at all.

Per-core layout trick: each core works on its adj block TRANSPOSED
([j=source node on partitions, i=own rows on free dim]) so that the
aggregate probs@h contracts over the partition dim as the tensor engine
requires. The transpose rides the DMA x-bar; adj is cast to fp16 host-side
(exact for a 0/1 mask, and halves HBM traffic).

The projection inputs are fed in fp16 (validated 4.2e-4 scale-relative
absmax on the final output): fp16 matmuls stream 2x faster and get fast
weight loads, and the w0/w1 columns are pre-halved on the host so the
tanh argument s/2 comes straight out of the projection matmul.

Schedule shape (engines are in-order; emission order seeds the queues):
  preamble -> [weights/features DMA | f1 path] -> h-projection batches,
  with the first activation groups' prep (adj transpose DMA, s=f1+f2,
  tanh, exp) interleaved as soon as their h batches drain -> steady
  pipeline: ACT runs tanh/exp back-to-back; DVE preadds+mask-muls; PE
  aggregates into 8 PSUM accumulators -> epilogue (denominator divide)
  and one batched output DMA.
"""

import os

import numpy as np

import concourse.mybir as mybir
import concourse.tile as tile
from concourse import bacc
from concourse.bass_utils import run_bass_kernel_spmd

F32 = mybir.dt.float32
F16 = mybir.dt.float16
AF = mybir.ActivationFunctionType

N, C_IN, C_OUT = 8192, 256, 128
NCORES = 8
ROWS = N // NCORES          # 1024 rows of adj per core
P = 128
NT = N // P                 # 64 node tiles (also the j-chunks)
NI = ROWS // P              # 8 output row-tiles per core
KC = [128, 128, 1]          # contraction chunks of K=257 (X.T rows + ones row)
WCOLS = C_OUT + 3           # [W | ones-hack | 0.5*w0 | 0.5*w1]
HCOLS = C_OUT + 1           # h plus the ones column
TINY = float(np.finfo(np.float32).tiny)
BANK = 512                  # PSUM bank, fp32 elements

# activation groups: j-chunks fused per tanh/exp instruction. The first two
# are small so the activation chain starts as early as possible (they only
# need the first h-projection batches); the last are small so the post-exp
# tail (mask-mul + matmul + epilogue) is short.
GROUPS = [2, 4] + [8] * 6 + [4, 4, 2]

_CACHE: dict = {}


def _build_nc(b_zero=True):
    nc = bacc.Bacc(
        "TRN2", target_bir_lowering=False, debug=False, num_devices=NCORES
    )
    xt1 = nc.dram_tensor("xt1", [257, N], F16, kind="ExternalInput").ap()
    xt1l = nc.dram_tensor("xt1l", [257, ROWS], F16, kind="ExternalInput").ap()
    wext = nc.dram_tensor("wext", [257, WCOLS], F16, kind="ExternalInput").ap()
    adjt = nc.dram_tensor("adjt", [N, ROWS], F16, kind="ExternalInput").ap()
    out = nc.dram_tensor("out", [ROWS, C_OUT], F32, kind="ExternalOutput").ap()

    with tile.TileContext(nc) as tc:
        _emit(tc, nc, xt1, xt1l, wext, adjt, out, b_zero)
    nc.compile()
    return nc


def _emit(tc, nc, xt1, xt1l, wext, adjt, out, b_zero):
    from contextlib import ExitStack

    # with b == 0 the K=1 "ones row" contraction chunk only contributes the
    # constant-one column of h_ext (done with a strided memset instead) and
    # zero constants to f1/f2 -- skip it entirely.
    nkc = 2 if b_zero else 3

    with ExitStack() as ctx:
        # ---- persistent tiles ----
        persist = ctx.enter_context(tc.tile_pool(name="persist", bufs=1))
        h16_all = persist.tile([P, NT * HCOLS], F16, tag="h16")   # [128, 8256]
        f2h_all = persist.tile([P, NT], F32, tag="f2h")           # 0.5*f2 per j
        f1rep = persist.tile([P, ROWS], F16, tag="f1rep")         # f1/2 bcast
        zero1 = persist.tile([P, 1], F32, tag="zero1")
        nc.vector.memset(zero1[:], 0.0)
        half1 = persist.tile([P, 1], F32, tag="half1")
        nc.vector.memset(half1[:], 0.5)
        if b_zero:
            # constant-one column of every h_ext tile (replaces the K=1
            # bias matmul chunk)
            nc.vector.memset(
                h16_all[:].rearrange("p (t c) -> p t c", c=HCOLS)[
                    :, :, C_OUT : C_OUT + 1
                ],
                1.0,
            )

        xtp = ctx.enter_context(tc.tile_pool(name="xt", bufs=1))

        # ---- input loads ----
        # small inputs first so the f1 path clears quickly. The xt sub-loads
        # are interleaved k0/k1 so the first node tiles have BOTH
        # contraction chunks resident as early as possible (tile dependency
        # tracking is AP-range based).
        offs = [0, 128, 256]
        xts = [
            xtp.tile([KC[k], N], F16, name=f"xtsb{k}", tag=f"xt{k}")
            for k in range(nkc)
        ]
        # sub-load column boundaries: a small leading slice covers the f2
        # head start and the first h batches, so it gates the whole
        # activation chain; later slices only need to keep ahead of the
        # h-projection loop
        SUBS = [0, 1024, 3072, 5120, N]
        # weights + local features first (they gate the longer f1 chain),
        # then the leading slice of both contraction halves
        wes, xls = [], []
        off = 0
        for k in range(nkc):
            kc = KC[k]
            wx_sb = xtp.tile([kc, WCOLS + ROWS], F16, name=f"wx{k}", tag=f"wx{k}")
            nc.sync.dma_start(wx_sb[:, 0:WCOLS], wext[off : off + kc, :])
            nc.sync.dma_start(wx_sb[:, WCOLS:], xt1l[off : off + kc, :])
            wes.append(wx_sb[:, 0:WCOLS])
            xls.append(wx_sb[:, WCOLS:])
            off += kc
        for k in range(nkc):
            if KC[k] == P:
                nc.sync.dma_start(
                    xts[k][:, 0 : SUBS[1]],
                    xt1[offs[k] : offs[k] + KC[k], 0 : SUBS[1]],
                )
        for c in range(1, len(SUBS) - 1):
            for k in range(nkc):
                if KC[k] != P:
                    if c == 1:
                        nc.sync.dma_start(
                            xts[k][:], xt1[offs[k] : offs[k] + KC[k], :]
                        )
                    continue
                nc.sync.dma_start(
                    xts[k][:, SUBS[c] : SUBS[c + 1]],
                    xt1[offs[k] : offs[k] + KC[k], SUBS[c] : SUBS[c + 1]],
                )

        # ---- f1 path: f1/2 for this core's rows, replicated across all
        # partitions directly by a matmul whose stationary operand is the
        # w0/2 column broadcast across the 128 PE columns ----
        with tc.tile_pool(name="pf", bufs=1, space="PSUM") as pfp:
            prep = pfp.tile([P, ROWS], F32, tag="prep")
            for k in range(nkc):
                for nh in range(ROWS // 512):
                    nc.tensor.matmul(
                        prep[:, nh * 512 : (nh + 1) * 512],
                        wes[k][:, C_OUT + 1 : C_OUT + 2].to_broadcast(
                            (KC[k], P)
                        ),
                        xls[k][:, nh * 512 : (nh + 1) * 512],
                        start=(k == 0),
                        stop=(k == nkc - 1),
                    )
            nc.scalar.copy(f1rep[:], prep[:])

        # ---- f2 head start: 0.5*f2 for the first 8 j-chunks via tiny
        # direct matmuls so activation groups 0/1 don't wait for the
        # h-projection pipeline ----
        F2HEAD = 8
        with tc.tile_pool(name="pf2", bufs=1, space="PSUM") as pf2p:
            pt = pf2p.tile([P, NI * BANK], F32, tag="pt")
            pt3 = pt[:].rearrange("p (t w) -> p t w", w=BANK)
            for q in range(F2HEAD):
                w = (q % NI) * BANK
                for k in range(nkc):
                    nc.tensor.matmul(
                        pt[:, w : w + 1],
                        xts[k][:, q * P : (q + 1) * P],
                        wes[k][:, C_OUT + 2 : C_OUT + 3],
                        start=(k == 0),
                        stop=(k == nkc - 1),
                    )
                if q == 1:
                    # group 0's two columns drain immediately so its
                    # preadds (and the whole activation chain) start early
                    nc.vector.tensor_copy(
                        f2h_all[:, 0:2], pt3[:, 0:2, 0:1]
                    )
            nc.vector.tensor_copy(
                f2h_all[:, 2:F2HEAD], pt3[:, 2:F2HEAD, 0:1]
            )

        # ---- main-loop pools (open before the h loop so activation groups
        # can be emitted interleaved with h batches) ----
        sup = ctx.enter_context(tc.tile_pool(name="sup", bufs=1))
        g16p = ctx.enter_context(tc.tile_pool(name="g16p", bufs=3))
        atp = ctx.enter_context(tc.tile_pool(name="atp", bufs=3))
        etp = ctx.enter_context(tc.tile_pool(name="etp", bufs=4))
        obp = ctx.enter_context(tc.tile_pool(name="ob", bufs=2))

        group_q0 = []
        q0 = 0
        for gsz in GROUPS:
            group_q0.append(q0)
            q0 += gsz

        deferred = []  # groups whose mask-mul+matmul emission is pending

        def emit_group_front(g):
            """adj transposes, s=f1+f2 preadds, fused tanh, fused exp."""
            gsz = GROUPS[g]
            q0 = group_q0[g]
            s_sup = sup.tile([P, gsz * ROWS], F16, tag="s", bufs=2, name=f"s{g}")
            g16 = g16p.tile([P, gsz * ROWS], F16, tag="g16", name=f"g16_{g}")
            at_sup = atp.tile(
                [P, gsz * ROWS], F16, tag="at", name=f"at{g}"
            )
            nc.sync.dma_start(
                at_sup[:].rearrange("p (q i) -> p q i", i=ROWS),
                adjt.rearrange("(q p) i -> p q i", p=P)[:, q0 : q0 + gsz, :],
            )
            for qq in range(gsz):
                q = q0 + qq
                # s = 0.5*f1_i + 0.5*f2_j   [j on partitions, i on free]
                nc.vector.tensor_scalar_add(
                    s_sup[:, qq * ROWS : (qq + 1) * ROWS],
                    f1rep[:],
                    f2h_all[:, q : q + 1],
                )
            # tanh in place (elementwise, same AP), then exp into g16:
            # exp(0.5*tanh + 0.5) = exp(sigmoid(s))
            nc.scalar.activation(s_sup[:], s_sup[:], AF.Tanh, bias=zero1[:])
            nc.scalar.activation(
                g16[:], s_sup[:], AF.Exp, bias=half1[:], scale=0.5
            )
            return {"g": g, "gsz": gsz, "q0": q0, "at": at_sup, "g16": g16}

        def emit_group_back(fr, pouts):
            """mask-mul + aggregate matmuls for a prepared group."""
            gsz, q0, at_sup, g16 = fr["gsz"], fr["q0"], fr["at"], fr["g16"]
            for qq in range(gsz):
                q = q0 + qq
                et = etp.tile([P, ROWS], F16, tag="et", name=f"et{q}")
                nc.vector.tensor_mul(
                    et[:],
                    at_sup[:, qq * ROWS : (qq + 1) * ROWS],
                    g16[:, qq * ROWS : (qq + 1) * ROWS],
                )
                rhs = h16_all[:, q * HCOLS : (q + 1) * HCOLS]
                for it in range(NI):
                    nc.tensor.matmul(
                        pouts[it],
                        et[:, it * P : (it + 1) * P],
                        rhs,
                        start=(q == 0),
                        stop=(q == NT - 1),
                    )

        # ---- h-projection: all 8 PSUM banks inside ONE tensor so four
        # tiles drain with a single strided copy. Pairs of node tiles have
        # their k-chunk matmuls interleaved so consecutive matmuls hit
        # different banks (same-bank accumulation serializes the PE). ----
        next_group = 0
        with tc.tile_pool(name="php", bufs=1, space="PSUM") as php:
            ph_all = php.tile([P, NI * BANK], F32, tag="ph")
            for b in range(NT // 4):  # batches of 4 node tiles
                for half in range(2):
                    nt0 = 4 * b + 2 * half
                    w0 = (nt0 % NI) * BANK
                    w1 = ((nt0 + 1) % NI) * BANK
                    for k in range(nkc):
                        nc.tensor.matmul(
                            ph_all[:, w0 : w0 + WCOLS],
                            xts[k][:, nt0 * P : (nt0 + 1) * P],
                            wes[k][:],
                            start=(k == 0),
                            stop=(k == nkc - 1),
                        )
                        nc.tensor.matmul(
                            ph_all[:, w1 : w1 + WCOLS],
                            xts[k][:, (nt0 + 1) * P : (nt0 + 2) * P],
                            wes[k][:],
                            start=(k == 0),
                            stop=(k == nkc - 1),
                        )
                # drain the 4 fresh tiles: h (+ones col) -> fp16, 0.5*f2 col
                bt = 4 * b
                wlo = (bt % NI) * BANK
                src = ph_all[:, wlo : wlo + 4 * BANK].rearrange(
                    "p (b w) -> p b w", b=4
                )
                dst_h = h16_all[:, bt * HCOLS : (bt + 4) * HCOLS].rearrange(
                    "p (b w) -> p b w", b=4
                )
                hc = C_OUT if b_zero else HCOLS
                nc.vector.tensor_copy(dst_h[:, :, 0:hc], src[:, :, 0:hc])
                if bt >= 8:  # first 8 f2 columns came from the head start
                    nc.vector.tensor_copy(
                        f2h_all[:, bt : bt + 4],
                        src[:, :, C_OUT + 2 : C_OUT + 3],
                    )
                # emit activation-group fronts as soon as their f2 columns
                # exist; their matmuls wait until the PSUM banks free up
                while (
                    next_group < len(GROUPS)
                    and group_q0[next_group] + GROUPS[next_group] <= 4 * (b + 1)
                    and len(deferred) < 3
                ):
                    deferred.append(emit_group_front(next_group))
                    next_group += 1

        # ---- aggregate accumulators: same 8 banks, next accumulation ----
        pop = ctx.enter_context(tc.tile_pool(name="po", bufs=1, space="PSUM"))
        po_all = pop.tile([P, NI * BANK], F32, tag="poall")
        pouts = [po_all[:, i * BANK : i * BANK + HCOLS] for i in range(NI)]

        # software-pipelined emission: keep group fronts (DVE preadds) one
        # group ahead of the backs (DVE mask-muls) so the in-order DVE
        # queue never starves the activation chain
        for g in range(next_group, len(GROUPS)):
            emit_group_back(deferred.pop(0), pouts)
            deferred.append(emit_group_front(g))
        for fr in deferred:
            emit_group_back(fr, pouts)

        # ---- epilogue: divide by clamped denominator, one batched store ----
        ob_all = obp.tile([P, NI * C_OUT], F32, tag="oball")
        po3 = po_all[:].rearrange("p (t w) -> p t w", w=BANK)
        dm = obp.tile([P, NI], F32, tag="dm")
        nc.vector.tensor_scalar_max(
            dm[:], po3[:, :, C_OUT : C_OUT + 1], TINY
        )
        rc = obp.tile([P, NI], F32, tag="rc")
        nc.vector.reciprocal(rc[:], dm[:])
        for it in range(NI):
            # alternate engines: ACT is idle after the last exp
            eng = nc.vector if it % 2 == 0 else nc.scalar
            if eng is nc.vector:
                nc.vector.tensor_scalar_mul(
                    ob_all[:, it * C_OUT : (it + 1) * C_OUT],
                    po_all[:, it * BANK : it * BANK + C_OUT],
                    rc[:, it : it + 1],
                )
            else:
                nc.scalar.mul(
                    ob_all[:, it * C_OUT : (it + 1) * C_OUT],
                    po_all[:, it * BANK : it * BANK + C_OUT],
                    rc[:, it : it + 1],
                )
        nc.sync.dma_start(
            out.rearrange("(t p) c -> p t c", p=P),
            ob_all[:].rearrange("p (t c) -> p t c", c=C_OUT),
        )


def _prep_inputs(node_feats, adj_matrix, W, b, v0, v1):
    X = np.ascontiguousarray(node_feats, dtype=np.float32)
    W = np.asarray(W, dtype=np.float32)
    b = np.asarray(b, dtype=np.float32)
    v0 = np.asarray(v0, dtype=np.float32)
    v1 = np.asarray(v1, dtype=np.float32)

    w0h = (0.5 * (W.astype(np.float64) @ v0.astype(np.float64))).astype(np.float32)
    w1h = (0.5 * (W.astype(np.float64) @ v1.astype(np.float64))).astype(np.float32)
    c0h = np.float32(0.5 * float(b.astype(np.float64) @ v0.astype(np.float64)))
    c1h = np.float32(0.5 * float(b.astype(np.float64) @ v1.astype(np.float64)))

    XT1 = np.empty((257, N), np.float32)
    XT1[:256] = X.T
    XT1[256] = 1.0

    WE = np.zeros((257, WCOLS), np.float32)
    WE[:256, :C_OUT] = W
    WE[256, :C_OUT] = b
    WE[256, C_OUT] = 1.0          # makes h_ext column 128 identically 1
    WE[:256, C_OUT + 1] = w0h
    WE[256, C_OUT + 1] = c0h
    WE[:256, C_OUT + 2] = w1h
    WE[256, C_OUT + 2] = c1h

    XT1h = XT1.astype(np.float16)
    WEh = WE.astype(np.float16)
    A16 = np.asarray(adj_matrix, dtype=np.float16)

    in_maps = []
    for c in range(NCORES):
        in_maps.append(
            {
                "xt1": XT1h,
                "xt1l": np.ascontiguousarray(XT1h[:, c * ROWS : (c + 1) * ROWS]),
                "wext": WEh,
                "adjt": np.ascontiguousarray(
                    A16[c * ROWS : (c + 1) * ROWS, :].T
                ),
            }
        )
    return in_maps


def _run(in_maps, trace=False, b_zero=True):
    key = f"nc_b{int(b_zero)}"
    if key not in _CACHE:
        _CACHE[key] = _build_nc(b_zero=b_zero)
    nc = _CACHE[key]
    res = run_bass_kernel_spmd(
        nc, in_maps, core_ids=list(range(NCORES)), trace=trace
    )
    full = np.concatenate(
        [res.results[c]["out"] for c in range(NCORES)], axis=0
    ).astype(np.float32)
    return full, res


def kernel(node_feats, adj_matrix, W, b, v0, v1):
    in_maps = _prep_inputs(node_feats, adj_matrix, W, b, v0, v1)
    trace = bool(int(os.environ.get("GAT_TRACE", "0")))
    b_zero = not bool(np.any(np.asarray(b)))
    full, _ = _run(in_maps, trace=trace, b_zero=b_zero)
    return full


# revision 55
# speedup vs baseline: 1.0133x; 1.0133x over previous
"""GAT single-head forward on 8 Trainium2 NeuronCores (Bass/Tile).

Math (per reference):
    h   = X @ W + b                      [N, 128]
    f1  = h @ v0, f2 = h @ v1            [N]
    logits = adj * (f1[:,None] + f2[None,:])   (adj entries are exactly 0/1)
    vals = sigmoid(logits) - 0.5
    masked softmax over row edges; out = probs @ h

Key identities used on device:
  * On edges (adj==1): val = sigmoid(f1_i + f2_j) - 0.5 in (-0.5, 0.5), so the
    softmax max-subtraction is unnecessary (exp never overflows) and constant
    shifts cancel:  probs = adj*exp(sigmoid(s)) / rowsum(adj*exp(sigmoid(s))).
  * sigmoid(s) = 0.5*tanh(s/2) + 0.5, so exp(sigmoid(s)) = exp(0.5*t + 0.5)
    with t = tanh(s/2). Tanh and Exp live in the SAME activation table set
    ("exp_and_others"), avoiding per-tile ~2.7us table swaps that
    Sigmoid+Exp would incur.
  * A ones-column appended to h turns the softmax denominator into one extra
    matmul output column (no separate row-reduction pass).

Sharding: rows of adj across the 8 cores (1024 rows each). node_feats is
small (8 MB) and is replicated, so every core computes the full projected
h locally - no collectives at all.

Per-core layout trick: each core works on its adj block TRANSPOSED
([j=source node on partitions, i=own rows on free dim]) so that the
aggregate probs@h contracts over the partition dim as the tensor engine
requires. The transpose rides the DMA x-bar; adj is cast to fp16 host-side
(exact for a 0/1 mask, and halves HBM traffic).

The projection inputs are fed in fp16 (validated 4.2e-4 scale-relative
absmax on the final output): fp16 matmuls stream 2x faster and get fast
weight loads, and the w0/w1 columns are pre-halved on the host so the
tanh argument s/2 comes straight out of the projection matmul.

Schedule shape (engines are in-order; emission order seeds the queues):
  preamble -> [weights/features DMA | f1 path] -> h-projection batches,
  with the first activation groups' prep (adj transpose DMA, s=f1+f2,
  tanh, exp) interleaved as soon as their h batches drain -> steady
  pipeline: ACT runs tanh/exp back-to-back; DVE preadds+mask-muls; PE
  aggregates into 8 PSUM accumulators -> epilogue (denominator divide)
  and one batched output DMA.
"""

import os

import numpy as np

import concourse.mybir as mybir
import concourse.tile as tile
from concourse import bacc
from concourse.bass_utils import run_bass_kernel_spmd

F32 = mybir.dt.float32
F16 = mybir.dt.float16
AF = mybir.ActivationFunctionType

N, C_IN, C_OUT = 8192, 256, 128
NCORES = 8
ROWS = N // NCORES          # 1024 rows of adj per core
P = 128
NT = N // P                 # 64 node tiles (also the j-chunks)
NI = ROWS // P              # 8 output row-tiles per core
KC = [128, 128, 1]          # contraction chunks of K=257 (X.T rows + ones row)
WCOLS = C_OUT + 3           # [W | ones-hack | 0.5*w0 | 0.5*w1]
HCOLS = C_OUT + 1           # h plus the ones column
TINY = float(np.finfo(np.float32).tiny)
BANK = 512                  # PSUM bank, fp32 elements

# activation groups: j-chunks fused per tanh/exp instruction. The first two
# are small so the activation chain starts as early as possible (they only
# need the first h-projection batches); the last are small so the post-exp
# tail (mask-mul + matmul + epilogue) is short.
GROUPS = [2, 4] + [8] * 6 + [4, 4, 2]

_CACHE: dict = {}


def _build_nc(b_zero=True):
    nc = bacc.Bacc(
        "TRN2", target_bir_lowering=False, debug=False, num_devices=NCORES
    )
    xt1 = nc.dram_tensor("xt1", [257, N], F16, kind="ExternalInput").ap()
    xt1l = nc.dram_tensor("xt1l", [257, ROWS], F16, kind="ExternalInput").ap()
    wext = nc.dram_tensor("wext", [257, WCOLS], F16, kind="ExternalInput").ap()
    adjt = nc.dram_tensor("adjt", [N, ROWS], F16, kind="ExternalInput").ap()
    out = nc.dram_tensor("out", [ROWS, C_OUT], F32, kind="ExternalOutput").ap()

    with tile.TileContext(nc) as tc:
        _emit(tc, nc, xt1, xt1l, wext, adjt, out, b_zero)
    nc.compile()
    return nc


def _emit(tc, nc, xt1, xt1l, wext, adjt, out, b_zero):
    from contextlib import ExitStack

    # with b == 0 the K=1 "ones row" contraction chunk only contributes the
    # constant-one column of h_ext (done with a strided memset instead) and
    # zero constants to f1/f2 -- skip it entirely.
    nkc = 2 if b_zero else 3

    with ExitStack() as ctx:
        # ---- persistent tiles ----
        persist = ctx.enter_context(tc.tile_pool(name="persist", bufs=1))
        h16_all = persist.tile([P, NT * HCOLS], F16, tag="h16")   # [128, 8256]
        f2h_all = persist.tile([P, NT], F32, tag="f2h")           # 0.5*f2 per j
        f1rep = persist.tile([P, ROWS], F16, tag="f1rep")         # f1/2 bcast
        zero1 = persist.tile([P, 1], F32, tag="zero1")
        nc.vector.memset(zero1[:], 0.0)
        half1 = persist.tile([P, 1], F32, tag="half1")
        nc.vector.memset(half1[:], 0.5)
        if b_zero:
            # constant-one column of every h_ext tile (replaces the K=1
            # bias matmul chunk)
            nc.vector.memset(
                h16_all[:].rearrange("p (t c) -> p t c", c=HCOLS)[
                    :, :, C_OUT : C_OUT + 1
                ],
                1.0,
            )

        xtp = ctx.enter_context(tc.tile_pool(name="xt", bufs=1))

        # ---- input loads ----
        # small inputs first so the f1 path clears quickly. The xt sub-loads
        # are interleaved k0/k1 so the first node tiles have BOTH
        # contraction chunks resident as early as possible (tile dependency
        # tracking is AP-range based).
        offs = [0, 128, 256]
        xts = [
            xtp.tile([KC[k], N], F16, name=f"xtsb{k}", tag=f"xt{k}")
            for k in range(nkc)
        ]
        # sub-load column boundaries: a small leading slice covers the f2
        # head start and the first h batches, so it gates the whole
        # activation chain; later slices only need to keep ahead of the
        # h-projection loop
        SUBS = [0, 1024, 3072, 5120, N]
        # weights + local features first (they gate the longer f1 chain),
        # then the leading slice of both contraction halves
        wes, xls = [], []
        off = 0
        for k in range(nkc):
            kc = KC[k]
            wx_sb = xtp.tile([kc, WCOLS + ROWS], F16, name=f"wx{k}", tag=f"wx{k}")
            nc.sync.dma_start(wx_sb[:, 0:WCOLS], wext[off : off + kc, :])
            nc.sync.dma_start(wx_sb[:, WCOLS:], xt1l[off : off + kc, :])
            wes.append(wx_sb[:, 0:WCOLS])
            xls.append(wx_sb[:, WCOLS:])
            off += kc
        for k in range(nkc):
            if KC[k] == P:
                nc.sync.dma_start(
                    xts[k][:, 0 : SUBS[1]],
                    xt1[offs[k] : offs[k] + KC[k], 0 : SUBS[1]],
                )
        for c in range(1, len(SUBS) - 1):
            for k in range(nkc):
                if KC[k] != P:
                    if c == 1:
                        nc.sync.dma_start(
                            xts[k][:], xt1[offs[k] : offs[k] + KC[k], :]
                        )
                    continue
                nc.sync.dma_start(
                    xts[k][:, SUBS[c] : SUBS[c + 1]],
                    xt1[offs[k] : offs[k] + KC[k], SUBS[c] : SUBS[c + 1]],
                )

        # ---- f1 path: f1/2 for this core's rows, replicated across all
        # partitions directly by a matmul whose stationary operand is the
        # w0/2 column broadcast across the 128 PE columns ----
        with tc.tile_pool(name="pf", bufs=1, space="PSUM") as pfp:
            prep = pfp.tile([P, ROWS], F32, tag="prep")
            for k in range(nkc):
                for nh in range(ROWS // 512):
                    nc.tensor.matmul(
                        prep[:, nh * 512 : (nh + 1) * 512],
                        wes[k][:, C_OUT + 1 : C_OUT + 2].to_broadcast(
                            (KC[k], P)
                        ),
                        xls[k][:, nh * 512 : (nh + 1) * 512],
                        start=(k == 0),
                        stop=(k == nkc - 1),
                    )
            nc.scalar.copy(f1rep[:], prep[:])

        # ---- f2 head start: 0.5*f2 for the first 8 j-chunks via tiny
        # direct matmuls so activation groups 0/1 don't wait for the
        # h-projection pipeline ----
        F2HEAD = 8
        with tc.tile_pool(name="pf2", bufs=1, space="PSUM") as pf2p:
            pt = pf2p.tile([P, NI * BANK], F32, tag="pt")
            pt3 = pt[:].rearrange("p (t w) -> p t w", w=BANK)
            for q in range(F2HEAD):
                w = (q % NI) * BANK
                for k in range(nkc):
                    nc.tensor.matmul(
                        pt[:, w : w + 1],
                        xts[k][:, q * P : (q + 1) * P],
                        wes[k][:, C_OUT + 2 : C_OUT + 3],
                        start=(k == 0),
                        stop=(k == nkc - 1),
                    )
                if q == 1:
                    # group 0's two columns drain immediately so its
                    # preadds (and the whole activation chain) start early
                    nc.vector.tensor_copy(
                        f2h_all[:, 0:2], pt3[:, 0:2, 0:1]
                    )
            nc.vector.tensor_copy(
                f2h_all[:, 2:F2HEAD], pt3[:, 2:F2HEAD, 0:1]
            )

        # ---- main-loop pools (open before the h loop so activation groups
        # can be emitted interleaved with h batches) ----
        sup = ctx.enter_context(tc.tile_pool(name="sup", bufs=1))
        g16p = ctx.enter_context(tc.tile_pool(name="g16p", bufs=3))
        atp = ctx.enter_context(tc.tile_pool(name="atp", bufs=3))
        etp = ctx.enter_context(tc.tile_pool(name="etp", bufs=4))
        obp = ctx.enter_context(tc.tile_pool(name="ob", bufs=2))

        group_q0 = []
        q0 = 0
        for gsz in GROUPS:
            group_q0.append(q0)
            q0 += gsz

        deferred = []  # groups whose mask-mul+matmul emission is pending

        def emit_group_front(g):
            """adj transposes, s=f1+f2 preadds, fused tanh, fused exp."""
            gsz = GROUPS[g]
            q0 = group_q0[g]
            s_sup = sup.tile([P, gsz * ROWS], F16, tag="s", bufs=2, name=f"s{g}")
            g16 = g16p.tile([P, gsz * ROWS], F16, tag="g16", name=f"g16_{g}")
            at_sup = atp.tile(
                [P, gsz * ROWS], F16, tag="at", name=f"at{g}"
            )
            nc.sync.dma_start(
                at_sup[:].rearrange("p (q i) -> p q i", i=ROWS),
                adjt.rearrange("(q p) i -> p q i", p=P)[:, q0 : q0 + gsz, :],
            )
            for qq in range(gsz):
                q = q0 + qq
                # s = 0.5*f1_i + 0.5*f2_j   [j on partitions, i on free]
                nc.vector.tensor_scalar_add(
                    s_sup[:, qq * ROWS : (qq + 1) * ROWS],
                    f1rep[:],
                    f2h_all[:, q : q + 1],
                )
            # tanh in place (elementwise, same AP), then exp into g16:
            # exp(0.5*tanh + 0.5) = exp(sigmoid(s))
            nc.scalar.activation(s_sup[:], s_sup[:], AF.Tanh, bias=zero1[:])
            nc.scalar.activation(
                g16[:], s_sup[:], AF.Exp, bias=half1[:], scale=0.5
            )
            return {"g": g, "gsz": gsz, "q0": q0, "at": at_sup, "g16": g16}

        def emit_group_back(fr, pouts, mid=None):
            """mask-mul + aggregate matmuls for a prepared group. `mid`
            emits the NEXT group's front after two mask-muls, so its
            preadds sit early in the in-order DVE queue instead of behind
            this whole burst (the remaining source of activation gaps)."""
            gsz, q0, at_sup, g16 = fr["gsz"], fr["q0"], fr["at"], fr["g16"]
            for qq in range(gsz):
                if qq == min(2, gsz - 1) and mid is not None:
                    mid()
                q = q0 + qq
                et = etp.tile([P, ROWS], F16, tag="et", name=f"et{q}")
                nc.vector.tensor_mul(
                    et[:],
                    at_sup[:, qq * ROWS : (qq + 1) * ROWS],
                    g16[:, qq * ROWS : (qq + 1) * ROWS],
                )
                rhs = h16_all[:, q * HCOLS : (q + 1) * HCOLS]
                for it in range(NI):
                    nc.tensor.matmul(
                        pouts[it],
                        et[:, it * P : (it + 1) * P],
                        rhs,
                        start=(q == 0),
                        stop=(q == NT - 1),
                    )

        # ---- h-projection: all 8 PSUM banks inside ONE tensor so four
        # tiles drain with a single strided copy. Pairs of node tiles have
        # their k-chunk matmuls interleaved so consecutive matmuls hit
        # different banks (same-bank accumulation serializes the PE). ----
        next_group = 0
        with tc.tile_pool(name="php", bufs=1, space="PSUM") as php:
            ph_all = php.tile([P, NI * BANK], F32, tag="ph")
            for b in range(NT // 4):  # batches of 4 node tiles
                for half in range(2):
                    nt0 = 4 * b + 2 * half
                    w0 = (nt0 % NI) * BANK
                    w1 = ((nt0 + 1) % NI) * BANK
                    for k in range(nkc):
                        nc.tensor.matmul(
                            ph_all[:, w0 : w0 + WCOLS],
                            xts[k][:, nt0 * P : (nt0 + 1) * P],
                            wes[k][:],
                            start=(k == 0),
                            stop=(k == nkc - 1),
                        )
                        nc.tensor.matmul(
                            ph_all[:, w1 : w1 + WCOLS],
                            xts[k][:, (nt0 + 1) * P : (nt0 + 2) * P],
                            wes[k][:],
                            start=(k == 0),
                            stop=(k == nkc - 1),
                        )
                # drain the 4 fresh tiles: h (+ones col) -> fp16, 0.5*f2 col
                bt = 4 * b
                wlo = (bt % NI) * BANK
                src = ph_all[:, wlo : wlo + 4 * BANK].rearrange(
                    "p (b w) -> p b w", b=4
                )
                dst_h = h16_all[:, bt * HCOLS : (bt + 4) * HCOLS].rearrange(
                    "p (b w) -> p b w", b=4
                )
                hc = C_OUT if b_zero else HCOLS
                nc.vector.tensor_copy(dst_h[:, :, 0:hc], src[:, :, 0:hc])
                if bt >= 8:  # first 8 f2 columns came from the head start
                    nc.vector.tensor_copy(
                        f2h_all[:, bt : bt + 4],
                        src[:, :, C_OUT + 2 : C_OUT + 3],
                    )
                # emit activation-group fronts as soon as their f2 columns
                # exist; their matmuls wait until the PSUM banks free up
                while (
                    next_group < len(GROUPS)
                    and group_q0[next_group] + GROUPS[next_group] <= 4 * (b + 1)
                    and len(deferred) < 3
                ):
                    deferred.append(emit_group_front(next_group))
                    next_group += 1

        # ---- aggregate accumulators: same 8 banks, next accumulation ----
        pop = ctx.enter_context(tc.tile_pool(name="po", bufs=1, space="PSUM"))
        po_all = pop.tile([P, NI * BANK], F32, tag="poall")
        pouts = [po_all[:, i * BANK : i * BANK + HCOLS] for i in range(NI)]

        # software-pipelined emission: keep group fronts (DVE preadds) one
        # group ahead of the backs (DVE mask-muls) so the in-order DVE
        # queue never starves the activation chain
        for g in range(next_group, len(GROUPS)):
            emit_group_back(
                deferred.pop(0),
                pouts,
                mid=lambda g=g: deferred.append(emit_group_front(g)),
            )
        for fr in deferred:
            emit_group_back(fr, pouts)

        # ---- epilogue: divide by clamped denominator, one batched store ----
        ob_all = obp.tile([P, NI * C_OUT], F32, tag="oball")
        po3 = po_all[:].rearrange("p (t w) -> p t w", w=BANK)
        dm = obp.tile([P, NI], F32, tag="dm")
        nc.vector.tensor_scalar_max(
            dm[:], po3[:, :, C_OUT : C_OUT + 1], TINY
        )
        rc = obp.tile([P, NI], F32, tag="rc")
        nc.vector.reciprocal(rc[:], dm[:])
        for it in range(NI):
            # alternate engines: ACT is idle after the last exp
            eng = nc.vector if it % 2 == 0 else nc.scalar
            if eng is nc.vector:
                nc.vector.tensor_scalar_mul(
                    ob_all[:, it * C_OUT : (it + 1) * C_OUT],
                    po_all[:, it * BANK : it * BANK + C_OUT],
                    rc[:, it : it + 1],
                )
            else:
                nc.scalar.mul(
                    ob_all[:, it * C_OUT : (it + 1) * C_OUT],
                    po_all[:, it * BANK : it * BANK + C_OUT],
                    rc[:, it : it + 1],
                )
        nc.sync.dma_start(
            out.rearrange("(t p) c -> p t c", p=P),
            ob_all[:].rearrange("p (t c) -> p t c", c=C_OUT),
        )


def _prep_inputs(node_feats, adj_matrix, W, b, v0, v1):
    X = np.ascontiguousarray(node_feats, dtype=np.float32)
    W = np.asarray(W, dtype=np.float32)
    b = np.asarray(b, dtype=np.float32)
    v0 = np.asarray(v0, dtype=np.float32)
    v1 = np.asarray(v1, dtype=np.float32)

    w0h = (0.5 * (W.astype(np.float64) @ v0.astype(np.float64))).astype(np.float32)
    w1h = (0.5 * (W.astype(np.float64) @ v1.astype(np.float64))).astype(np.float32)
    c0h = np.float32(0.5 * float(b.astype(np.float64) @ v0.astype(np.float64)))
    c1h = np.float32(0.5 * float(b.astype(np.float64) @ v1.astype(np.float64)))

    XT1 = np.empty((257, N), np.float32)
    XT1[:256] = X.T
    XT1[256] = 1.0

    WE = np.zeros((257, WCOLS), np.float32)
    WE[:256, :C_OUT] = W
    WE[256, :C_OUT] = b
    WE[256, C_OUT] = 1.0          # makes h_ext column 128 identically 1
    WE[:256, C_OUT + 1] = w0h
    WE[256, C_OUT + 1] = c0h
    WE[:256, C_OUT + 2] = w1h
    WE[256, C_OUT + 2] = c1h

    XT1h = XT1.astype(np.float16)
    WEh = WE.astype(np.float16)
    A16 = np.asarray(adj_matrix, dtype=np.float16)

    in_maps = []
    for c in range(NCORES):
        in_maps.append(
            {
                "xt1": XT1h,
                "xt1l": np.ascontiguousarray(XT1h[:, c * ROWS : (c + 1) * ROWS]),
                "wext": WEh,
                "adjt": np.ascontiguousarray(
                    A16[c * ROWS : (c + 1) * ROWS, :].T
                ),
            }
        )
    return in_maps


def _run(in_maps, trace=False, b_zero=True):
    key = f"nc_b{int(b_zero)}"
    if key not in _CACHE:
        _CACHE[key] = _build_nc(b_zero=b_zero)
    nc = _CACHE[key]
    res = run_bass_kernel_spmd(
        nc, in_maps, core_ids=list(range(NCORES)), trace=trace
    )
    full = np.concatenate(
        [res.results[c]["out"] for c in range(NCORES)], axis=0
    ).astype(np.float32)
    return full, res


def kernel(node_feats, adj_matrix, W, b, v0, v1):
    in_maps = _prep_inputs(node_feats, adj_matrix, W, b, v0, v1)
    trace = bool(int(os.environ.get("GAT_TRACE", "0")))
    b_zero = not bool(np.any(np.asarray(b)))
    full, _ = _run(in_maps, trace=trace, b_zero=b_zero)
    return full
